# revision 1
# baseline (speedup 1.0000x reference)
"""Trainium2 Bass kernel for nn_Critic (gnn_message_passing).

Sharding: node-sharded across 8 NeuronCores (expert-style, weights are
per-node banks). Rank r owns nodes [32r, 32r+32).

Phase 1 (per rank, 32 nodes): per-node 2-layer MLPs in feature-major
layout (stationary operand = weight tiles, moving = host-pre-transposed
obs/act), fp32r matmuls (full PE rate at free-dim 256, ~1e-3 max rel
err), ReLU+bias fused on ScalarE reading PSUM. V and A layer-2 results
accumulate into one PSUM tile so Q = V + A is free. Q^T [64,256] per
node goes to a DRAM shard laid out node-major.

AllGather: 2 MB/rank -> 16 MB full Q, node-major so Q of node j is the
contiguous 64KB row-block [64j : 64j+64) of a [16384, 256] buffer.

Phase 2 (per node-pair): one SWDGE dma_gather pulls 640 rows (4
neighbors + center, two nodes stacked 64+64 on partitions); 11 DVE min
ops build all 15 subset minima; 16 accumulating PE matmuls against
host-built [128,2] weight columns (Mobius chi coefficients with /HEADS
and the mean-over-F folded in) produce both nodes' outputs in a [2,256]
PSUM tile.

The program is SPMD-uniform and input-value-independent: all
rank-dependent data (shards, gather indices, chi weight columns)
arrives as per-rank input tensors, so the NEFF is built once and
cached.
"""
import sys

if "/opt/trn_rl_repo" not in sys.path:
    sys.path.insert(0, "/opt/trn_rl_repo")

import numpy as np

import concourse.bacc as bacc
import concourse.mybir as mybir
import concourse.tile as tile
from concourse.bass_utils import run_bass_kernel_spmd

B, N, H, F = 256, 256, 256, 64
NNB = 4
HEADS = 3
S = 2**NNB - 1            # 15 nonempty subsets
NCORES = 8
NL = N // NCORES          # 32 nodes per rank
NPAIR = NL // 2           # 16 node pairs per rank
NG = NNB + 1              # gathered blocks per pair: 4 neighbors + center
NIDX = NG * 128           # 640 gather rows per pair
IDXC = NIDX // 16         # 40 idx columns per pair

F32 = mybir.dt.float32
F32R = mybir.dt.float32r
I16 = mybir.dt.int16

# composite subsets (popcount >= 2), mask order; M[s] = min(M[s^low], T[lowbit])
_COMPOSITE = [s for s in range(1, S + 1) if bin(s).count("1") >= 2]
_SLOT = {s: i for i, s in enumerate(_COMPOSITE)}  # mask -> column slot in mt tile


def _build_program():
    nc = bacc.Bacc("TRN2", target_bir_lowering=False, debug=False,
                   num_devices=NCORES)

    obsT = nc.dram_tensor("obsT", [NL, H, B], F32, kind="ExternalInput")
    actT = nc.dram_tensor("actT", [NL, H, B], F32, kind="ExternalInput")
    vw1 = nc.dram_tensor("vw1", [NL, H, H], F32, kind="ExternalInput")
    aw1 = nc.dram_tensor("aw1", [NL, 2 * H, H], F32, kind="ExternalInput")
    vw2 = nc.dram_tensor("vw2", [NL, H, F], F32, kind="ExternalInput")
    aw2 = nc.dram_tensor("aw2", [NL, H, F], F32, kind="ExternalInput")
    vb1T = nc.dram_tensor("vb1T", [H, NL], F32, kind="ExternalInput")
    ab1T = nc.dram_tensor("ab1T", [H, NL], F32, kind="ExternalInput")
    vb2T = nc.dram_tensor("vb2T", [F, NL], F32, kind="ExternalInput")
    ab2T = nc.dram_tensor("ab2T", [F, NL], F32, kind="ExternalInput")
    ws = nc.dram_tensor("ws", [128, NPAIR * 32], F32, kind="ExternalInput")
    gidx = nc.dram_tensor("gidx", [128, NPAIR * IDXC], I16, kind="ExternalInput")
    out = nc.dram_tensor("out", [2, NPAIR, B], F32, kind="ExternalOutput")

    with tile.TileContext(nc) as tc:
        with (
            tc.tile_pool(name="const", bufs=1) as cpool,
            tc.tile_pool(name="io", bufs=3) as iopool,
            tc.tile_pool(name="w", bufs=3) as wpool,
            tc.tile_pool(name="h", bufs=2) as hpool,
            tc.tile_pool(name="q", bufs=3) as qpool,
            tc.tile_pool(name="g2", bufs=2) as gpool,
            tc.tile_pool(name="ps", bufs=2, space="PSUM") as ps,
            tc.tile_pool(name="dram", bufs=1, space="DRAM") as dram,
        ):
            # ---- constants
            b1v_sb = [cpool.tile([128, NL], F32, name=f"b1v{k}", tag=f"b1v{k}")
                      for k in range(2)]
            b1a_sb = [cpool.tile([128, NL], F32, name=f"b1a{k}", tag=f"b1a{k}")
                      for k in range(2)]
            for k in range(2):
                nc.sync.dma_start(b1v_sb[k][:], vb1T.ap()[128 * k:128 * (k + 1), :])
                nc.sync.dma_start(b1a_sb[k][:], ab1T.ap()[128 * k:128 * (k + 1), :])
            b2v_sb = cpool.tile([F, NL], F32, name="b2v", tag="b2v")
            b2a_sb = cpool.tile([F, NL], F32, name="b2a", tag="b2a")
            qb2_sb = cpool.tile([F, NL], F32, name="qb2", tag="qb2")
            nc.sync.dma_start(b2v_sb[:], vb2T.ap())
            nc.sync.dma_start(b2a_sb[:], ab2T.ap())
            nc.vector.tensor_add(qb2_sb[:], b2v_sb[:], b2a_sb[:])
            gidx_sb = cpool.tile([128, NPAIR * IDXC], I16, name="gidx_sb", tag="gidx_sb")
            nc.sync.dma_start(gidx_sb[:], gidx.ap())
            ws_sb = cpool.tile([128, NPAIR * 32], F32R, name="ws_sb", tag="ws_sb")
            nc.gpsimd.dma_start(ws_sb[:], ws.ap())  # cast fp32 -> fp32r
            stage = cpool.tile([2, NPAIR * B], F32, name="stage", tag="stage")

            qshard = dram.tile([NL * F, B], F32R, name="qshard")
            qfull = dram.tile([N * F, B], F32R, name="qfull")

            # ---- phase 1: per-node MLPs -> Q^T shard
            # HWDGE bitcast loads (no fp32->fp32r cast DMA): keeps the
            # GpSimd Q7 descriptor generator off the critical path (~1us
            # of serialized Q7 time per SWDGE dma_start), and merges each
            # operand's k-tiles into one DMA (6 DMAs/node instead of 15).
            def loadm(pool, kb, cols, src2d, nm, tg):
                t = pool.tile([128, kb * cols], F32R, name=nm, tag=tg)
                nc.sync.dma_start(
                    t[:].rearrange("p (k e) -> p k e", k=kb),
                    src2d.rearrange("(k p) e -> p k e", p=128).bitcast(F32R))
                return t

            for n in range(NL):
                oxt = loadm(iopool, 2, B, obsT.ap()[n], f"oxt_{n}", "oxt")
                axt = loadm(iopool, 2, B, actT.ap()[n], f"axt_{n}", "axt")
                w1vt = loadm(wpool, 2, H, vw1.ap()[n], f"w1vt_{n}", "w1vt")
                w1at = loadm(wpool, 4, H, aw1.ap()[n], f"w1at_{n}", "w1at")
                w2vt = loadm(wpool, 2, F, vw2.ap()[n], f"w2vt_{n}", "w2vt")
                w2at = loadm(wpool, 2, F, aw2.ap()[n], f"w2at_{n}", "w2at")
                ox = [oxt[:, B * k:B * (k + 1)] for k in range(2)]
                ax = [axt[:, B * k:B * (k + 1)] for k in range(2)]
                w1v = lambda k, osl: w1vt[:, H * k + osl.start:H * k + osl.stop]
                w1a = lambda k, osl: w1at[:, H * k + osl.start:H * k + osl.stop]
                w2v = [w2vt[:, F * k:F * (k + 1)] for k in range(2)]
                w2a = [w2at[:, F * k:F * (k + 1)] for k in range(2)]

                hv = []
                ha = []
                for o in range(2):
                    osl = slice(128 * o, 128 * (o + 1))
                    hpv = ps.tile([128, B], F32, name=f"hpv_{n}_{o}", tag="hpv")
                    nc.tensor.matmul(hpv[:], w1v(0, osl), ox[0], start=True, stop=False)
                    nc.tensor.matmul(hpv[:], w1v(1, osl), ox[1], start=False, stop=True)
                    hvt = hpool.tile([128, B], F32R, name=f"hv{o}_{n}", tag=f"hv{o}")
                    nc.scalar.activation(hvt[:], hpv[:],
                                         mybir.ActivationFunctionType.Relu,
                                         bias=b1v_sb[o][:, n:n + 1])
                    hv.append(hvt)

                    hpa = ps.tile([128, B], F32, name=f"hpa_{n}_{o}", tag="hpa")
                    nc.tensor.matmul(hpa[:], w1a(0, osl), ox[0], start=True, stop=False)
                    nc.tensor.matmul(hpa[:], w1a(1, osl), ox[1], start=False, stop=False)
                    nc.tensor.matmul(hpa[:], w1a(2, osl), ax[0], start=False, stop=False)
                    nc.tensor.matmul(hpa[:], w1a(3, osl), ax[1], start=False, stop=True)
                    hat = hpool.tile([128, B], F32R, name=f"ha{o}_{n}", tag=f"ha{o}")
                    nc.scalar.activation(hat[:], hpa[:],
                                         mybir.ActivationFunctionType.Relu,
                                         bias=b1a_sb[o][:, n:n + 1])
                    ha.append(hat)

                qp = ps.tile([F, B], F32, name=f"qp_{n}", tag="qp")
                nc.tensor.matmul(qp[:], w2v[0], hv[0][:], start=True, stop=False)
                nc.tensor.matmul(qp[:], w2v[1], hv[1][:], start=False, stop=False)
                nc.tensor.matmul(qp[:], w2a[0], ha[0][:], start=False, stop=False)
                nc.tensor.matmul(qp[:], w2a[1], ha[1][:], start=False, stop=True)
                qt = qpool.tile([F, B], F32R, name=f"qt_{n}", tag="qt")
                nc.scalar.activation(qt[:], qp[:],
                                     mybir.ActivationFunctionType.Identity,
                                     bias=qb2_sb[:, n:n + 1])
                nc.sync.dma_start(qshard[F * n:F * (n + 1), :], qt[:])

            # ---- exchange Q across ranks (node-major concat on axis 0)
            nc.gpsimd.collective_compute(
                "AllGather", mybir.AluOpType.bypass,
                replica_groups=[list(range(NCORES))],
                ins=[qshard.opt()], outs=[qfull.opt()],
            )

            # ---- phase 2: Choquet reduction per node pair
            for q in range(NPAIR):
                g = gpool.tile([128, NG * B], F32R, name=f"g_{q}", tag="g")
                nc.gpsimd.dma_gather(
                    g[:].rearrange("p (c e) -> p c e", e=B),
                    qfull[:],
                    gidx_sb[:, IDXC * q:IDXC * (q + 1)],
                    NIDX, NIDX, B,
                )

                def T(j):
                    return g[:, B * j:B * (j + 1)]

                mt = gpool.tile([128, len(_COMPOSITE) * B], F32R,
                                name=f"mt_{q}", tag="mt")

                def M(mask):
                    if bin(mask).count("1") == 1:
                        return T(mask.bit_length() - 1)
                    sl = _SLOT[mask]
                    return mt[:, B * sl:B * (sl + 1)]

                for mask in _COMPOSITE:
                    low = mask & (-mask)
                    nc.vector.tensor_tensor(
                        M(mask), M(mask ^ low), T(low.bit_length() - 1),
                        mybir.AluOpType.min)

                acc = ps.tile([2, B], F32, name=f"acc_{q}", tag="acc")
                for k in range(16):
                    rhs = T(NNB) if k == 15 else M(k + 1)
                    nc.tensor.matmul(acc[:], ws_sb[:, 32 * q + 2 * k:32 * q + 2 * k + 2],
                                     rhs, start=(k == 0), stop=(k == 15))
                nc.vector.tensor_copy(stage[:, B * q:B * (q + 1)], acc[:])

            nc.sync.dma_start(out.ap().rearrange("a q b -> a (q b)"), stage[:])

    nc.compile()
    return nc


_PROG = None


def _get_program():
    global _PROG
    if _PROG is None:
        _PROG = _build_program()
    return _PROG


def _shard_inputs(observation, action, V_W1, V_b1, V_W2, V_b2,
                  A_W1, A_b1, A_W2, A_b2, chi_m, local_edges):
    observation = np.asarray(observation, np.float32)
    action = np.asarray(action, np.float32)
    V_W1 = np.asarray(V_W1, np.float32)
    V_b1 = np.asarray(V_b1, np.float32)
    V_W2 = np.asarray(V_W2, np.float32)
    V_b2 = np.asarray(V_b2, np.float32)
    A_W1 = np.asarray(A_W1, np.float32)
    A_b1 = np.asarray(A_b1, np.float32)
    A_W2 = np.asarray(A_W2, np.float32)
    A_b2 = np.asarray(A_b2, np.float32)
    chi_m = np.asarray(chi_m, np.float32)
    local_edges = np.asarray(local_edges)

    centers = local_edges[:, 0, 0].astype(np.int64)
    neigh = local_edges[:, 0, 1:].astype(np.int64)
    csum = chi_m.sum(axis=1)  # [N, S]

    in_maps = []
    for r in range(NCORES):
        nodes = np.arange(r * NL, (r + 1) * NL)
        obsT = np.ascontiguousarray(observation[:, nodes, :].transpose(1, 2, 0))
        actT = np.ascontiguousarray(action[:, nodes, :].transpose(1, 2, 0))

        # PE weight columns for the Choquet weighted sum (+ mean over F)
        wsh = np.zeros((128, NPAIR * 32), np.float32)
        # gather indices, idx i of pair q at [16g + i%16, 40q + i//16]
        gidxh = np.zeros((128, NPAIR * IDXC), np.int16)
        for q in range(NPAIR):
            n0 = int(nodes[2 * q])
            n1 = int(nodes[2 * q + 1])
            for k in range(S):
                wsh[0:64, 32 * q + 2 * k] = csum[n0, k] / (HEADS * F)
                wsh[64:128, 32 * q + 2 * k + 1] = csum[n1, k] / (HEADS * F)
            wsh[0:64, 32 * q + 30] = 1.0 / F
            wsh[64:128, 32 * q + 31] = 1.0 / F

            rows = np.empty(NIDX, np.int16)
            p = np.arange(64)
            for j in range(NNB):
                rows[128 * j:128 * j + 64] = neigh[n0, j] * F + p
                rows[128 * j + 64:128 * (j + 1)] = neigh[n1, j] * F + p
            rows[128 * NNB:128 * NNB + 64] = centers[n0] * F + p
            rows[128 * NNB + 64:128 * NG] = centers[n1] * F + p
            i = np.arange(NIDX)
            for grp in range(8):
                gidxh[16 * grp + (i % 16), IDXC * q + i // 16] = rows

        in_maps.append({
            "obsT": obsT,
            "actT": actT,
            "vw1": np.ascontiguousarray(V_W1[nodes]),
            "aw1": np.ascontiguousarray(A_W1[nodes]),
            "vw2": np.ascontiguousarray(V_W2[nodes]),
            "aw2": np.ascontiguousarray(A_W2[nodes]),
            "vb1T": np.ascontiguousarray(V_b1[nodes].T),
            "ab1T": np.ascontiguousarray(A_b1[nodes].T),
            "vb2T": np.ascontiguousarray(V_b2[nodes].T),
            "ab2T": np.ascontiguousarray(A_b2[nodes].T),
            "ws": wsh,
            "gidx": gidxh,
        })
    return in_maps


def _unshard_output(results):
    out = np.empty((B, N), np.float32)
    for r in range(NCORES):
        o = results[r]["out"]  # [2, NPAIR, B]
        for q in range(NPAIR):
            out[:, r * NL + 2 * q] = o[0, q]
            out[:, r * NL + 2 * q + 1] = o[1, q]
    return out


def kernel(**inputs) -> np.ndarray:
    nc = _get_program()
    in_maps = _shard_inputs(**inputs)
    results = run_bass_kernel_spmd(nc, in_maps, list(range(NCORES))).results
    return _unshard_output(results)


if __name__ == "__main__":
    import jax

    rng = np.random.default_rng(0)
    ins = {
        "observation": rng.standard_normal((B, N, H)).astype(np.float32),
        "action": rng.standard_normal((B, N, H)).astype(np.float32),
        "V_W1": (rng.standard_normal((N, H, H)) * 0.02).astype(np.float32),
        "V_b1": rng.standard_normal((N, H)).astype(np.float32) * 0.1,
        "V_W2": (rng.standard_normal((N, H, F)) * 0.02).astype(np.float32),
        "V_b2": rng.standard_normal((N, F)).astype(np.float32) * 0.1,
        "A_W1": (rng.standard_normal((N, 2 * H, H)) * 0.02).astype(np.float32),
        "A_b1": rng.standard_normal((N, H)).astype(np.float32) * 0.1,
        "A_W2": (rng.standard_normal((N, H, F)) * 0.02).astype(np.float32),
        "A_b2": rng.standard_normal((N, F)).astype(np.float32) * 0.1,
        "chi_m": (rng.standard_normal((N, HEADS, S)) * 0.02).astype(np.float32),
        "local_edges": np.concatenate(
            [np.arange(N, dtype=np.int64)[:, None],
             rng.integers(0, N, (N, NNB)).astype(np.int64)], axis=1)[:, None, :],
    }
    got = kernel(**ins)
    print("kernel ran, out:", got.shape, got.dtype, got[:2, :4])



# revision 2
# speedup vs baseline: 1.1247x; 1.1247x over previous
"""Trainium2 Bass kernel for nn_Critic (gnn_message_passing) — v2.

Sharding: node-sharded across 8 NeuronCores. Rank r owns nodes
[32r, 32r+32).

v2 over baseline:
- bf16 end-to-end (weights/activations/Q exchange/mins; fp32 PSUM and
  biases). Halves HBM traffic, PE time, AllGather bytes and DVE work.
  Verified 4.1e-3 rel err vs fp32 reference (gate 2e-2).
- Host pre-swizzles every phase-1 operand so each DMA line is one
  contiguous 1-2KB run per partition (single fully-linear descriptor
  pattern per load).
- AllGather output lives in a Shared-addr-space DRAM tensor (required
  for the one-hop collective path; non-Shared bounces via scratchpad).

Phase 2 (unchanged structure): per node-pair SWDGE gather of 640 rows
(4 neighbors + center x 2 nodes stacked 64+64 on partitions), 11 DVE
mins for the 15 subset minima, 16 accumulating PE matmuls against
host-built [128,2] chi columns (Mobius coefficients with /HEADS and
mean-over-F folded in) -> [2,256] PSUM per pair.
"""
import sys

if "/opt/trn_rl_repo" not in sys.path:
    sys.path.insert(0, "/opt/trn_rl_repo")

import numpy as np
import ml_dtypes

import concourse.bacc as bacc
import concourse.mybir as mybir
import concourse.tile as tile
from concourse.bass_utils import run_bass_kernel_spmd

B, N, H, F = 256, 256, 256, 64
NNB = 4
HEADS = 3
S = 2**NNB - 1            # 15 nonempty subsets
NCORES = 8
NL = N // NCORES          # 32 nodes per rank
NPAIR = NL // 2           # 16 node pairs per rank
NG = NNB + 1              # gathered blocks per pair: 4 neighbors + center
NIDX = NG * 128           # 640 gather rows per pair
IDXC = NIDX // 16         # 40 idx columns per pair
CH = 4                    # AllGather chunks
NCH = NL // CH            # 8 nodes per chunk
CROWS = NCH * F           # 512 qshard rows per chunk
GROWS = NCORES * CROWS    # 4096 qfull rows per chunk

F32 = mybir.dt.float32
BF16 = mybir.dt.bfloat16
I16 = mybir.dt.int16
NPBF = ml_dtypes.bfloat16

# composite subsets (popcount >= 2), mask order; M[s] = min(M[s^low], T[lowbit])
_COMPOSITE = [s for s in range(1, S + 1) if bin(s).count("1") >= 2]
_SLOT = {s: i for i, s in enumerate(_COMPOSITE)}


def _build_program():
    nc = bacc.Bacc("TRN2", target_bir_lowering=False, debug=False,
                   num_devices=NCORES)

    # one packed per-node operand bank: [oxt|axt|w1vt|w1at|w2vt|w2at]
    # on the free axis -> a single 128x5.5KB DMA per node instead of 6
    # (HWDGE is fixed-cost-per-DMA bound, ~630ns each).
    PKC = 2 * B + 2 * B + 2 * H + 4 * H + 2 * F + 2 * F   # 2816 cols
    OX, AX, W1V, W1A, W2V, W2A = 0, 512, 1024, 1536, 2560, 2688
    pk = nc.dram_tensor("pk", [NL, 128, PKC], BF16, kind="ExternalInput")
    vb1T = nc.dram_tensor("vb1T", [H, NL], F32, kind="ExternalInput")
    ab1T = nc.dram_tensor("ab1T", [H, NL], F32, kind="ExternalInput")
    vb2T = nc.dram_tensor("vb2T", [F, NL], F32, kind="ExternalInput")
    ab2T = nc.dram_tensor("ab2T", [F, NL], F32, kind="ExternalInput")
    ws = nc.dram_tensor("ws", [128, NPAIR * 32], BF16, kind="ExternalInput")
    gidx = nc.dram_tensor("gidx", [128, NPAIR * IDXC], I16, kind="ExternalInput")
    out = nc.dram_tensor("out", [2, NPAIR, B], F32, kind="ExternalOutput")

    with tile.TileContext(nc) as tc:
        with (
            tc.tile_pool(name="const", bufs=1) as cpool,
            tc.tile_pool(name="io", bufs=3) as iopool,
            tc.tile_pool(name="w", bufs=3) as wpool,
            tc.tile_pool(name="h", bufs=2) as hpool,
            tc.tile_pool(name="q", bufs=3) as qpool,
            tc.tile_pool(name="g2", bufs=2) as gpool,
            tc.tile_pool(name="ps", bufs=2, space="PSUM") as ps,
            tc.tile_pool(name="dram", bufs=1, space="DRAM") as dram,
        ):
            # ---- constants
            b1v_sb = [cpool.tile([128, NL], F32, name=f"b1v{k}", tag=f"b1v{k}")
                      for k in range(2)]
            b1a_sb = [cpool.tile([128, NL], F32, name=f"b1a{k}", tag=f"b1a{k}")
                      for k in range(2)]
            for k in range(2):
                nc.sync.dma_start(b1v_sb[k][:], vb1T.ap()[128 * k:128 * (k + 1), :])
                nc.sync.dma_start(b1a_sb[k][:], ab1T.ap()[128 * k:128 * (k + 1), :])
            b2v_sb = cpool.tile([F, NL], F32, name="b2v", tag="b2v")
            b2a_sb = cpool.tile([F, NL], F32, name="b2a", tag="b2a")
            qb2_sb = cpool.tile([F, NL], F32, name="qb2", tag="qb2")
            nc.sync.dma_start(b2v_sb[:], vb2T.ap())
            nc.sync.dma_start(b2a_sb[:], ab2T.ap())
            nc.vector.tensor_add(qb2_sb[:], b2v_sb[:], b2a_sb[:])
            gidx_sb = cpool.tile([128, NPAIR * IDXC], I16, name="gidx_sb", tag="gidx_sb")
            nc.sync.dma_start(gidx_sb[:], gidx.ap())
            ws_sb = cpool.tile([128, NPAIR * 32], BF16, name="ws_sb", tag="ws_sb")
            nc.sync.dma_start(ws_sb[:], ws.ap())
            stage = cpool.tile([2, NPAIR * B], F32, name="stage", tag="stage")

            qshard = dram.tile([NL * F, B], BF16, name="qshard")
            qfull = dram.tile([N * F, B], BF16, name="qfull",
                              addr_space="Shared")

            # ---- phase 1: per-node MLPs -> Q^T shard
            for n in range(NL):
                pkt = wpool.tile([128, PKC], BF16, name=f"pk_{n}", tag="pk")
                nc.sync.dma_start(pkt[:], pk.ap()[n])
                ox = [pkt[:, OX + B * k:OX + B * (k + 1)] for k in range(2)]
                ax = [pkt[:, AX + B * k:AX + B * (k + 1)] for k in range(2)]
                w1v = lambda k, osl: pkt[:, W1V + H * k + osl.start:W1V + H * k + osl.stop]
                w1a = lambda k, osl: pkt[:, W1A + H * k + osl.start:W1A + H * k + osl.stop]
                w2v = [pkt[:, W2V + F * k:W2V + F * (k + 1)] for k in range(2)]
                w2a = [pkt[:, W2A + F * k:W2A + F * (k + 1)] for k in range(2)]

                hv = []
                ha = []
                for o in range(2):
                    osl = slice(128 * o, 128 * (o + 1))
                    hpv = ps.tile([128, B], F32, name=f"hpv_{n}_{o}", tag="hpv")
                    nc.tensor.matmul(hpv[:], w1v(0, osl), ox[0], start=True, stop=False)
                    nc.tensor.matmul(hpv[:], w1v(1, osl), ox[1], start=False, stop=True)
                    hvt = hpool.tile([128, B], BF16, name=f"hv{o}_{n}", tag=f"hv{o}")
                    nc.scalar.activation(hvt[:], hpv[:],
                                         mybir.ActivationFunctionType.Relu,
                                         bias=b1v_sb[o][:, n:n + 1])
                    hv.append(hvt)

                    hpa = ps.tile([128, B], F32, name=f"hpa_{n}_{o}", tag="hpa")
                    nc.tensor.matmul(hpa[:], w1a(0, osl), ox[0], start=True, stop=False)
                    nc.tensor.matmul(hpa[:], w1a(1, osl), ox[1], start=False, stop=False)
                    nc.tensor.matmul(hpa[:], w1a(2, osl), ax[0], start=False, stop=False)
                    nc.tensor.matmul(hpa[:], w1a(3, osl), ax[1], start=False, stop=True)
                    hat = hpool.tile([128, B], BF16, name=f"ha{o}_{n}", tag=f"ha{o}")
                    nc.scalar.activation(hat[:], hpa[:],
                                         mybir.ActivationFunctionType.Relu,
                                         bias=b1a_sb[o][:, n:n + 1])
                    ha.append(hat)

                qp = ps.tile([F, B], F32, name=f"qp_{n}", tag="qp")
                nc.tensor.matmul(qp[:], w2v[0], hv[0][:], start=True, stop=False)
                nc.tensor.matmul(qp[:], w2v[1], hv[1][:], start=False, stop=False)
                nc.tensor.matmul(qp[:], w2a[0], ha[0][:], start=False, stop=False)
                nc.tensor.matmul(qp[:], w2a[1], ha[1][:], start=False, stop=True)
                qt = qpool.tile([F, B], BF16, name=f"qt_{n}", tag="qt")
                nc.scalar.activation(qt[:], qp[:],
                                     mybir.ActivationFunctionType.Identity,
                                     bias=qb2_sb[:, n:n + 1])
                nc.sync.dma_start(qshard[F * n:F * (n + 1), :], qt[:])

            # ---- exchange Q across ranks (node-major concat on axis 0)
            nc.gpsimd.collective_compute(
                "AllGather", mybir.AluOpType.bypass,
                replica_groups=[list(range(NCORES))],
                ins=[qshard.opt()], outs=[qfull.opt()],
            )

            # ---- phase 2: Choquet reduction per node pair
            for q in range(NPAIR):
                g = gpool.tile([128, NG * B], BF16, name=f"g_{q}", tag="g")
                nc.gpsimd.dma_gather(
                    g[:].rearrange("p (c e) -> p c e", e=B),
                    qfull[:],
                    gidx_sb[:, IDXC * q:IDXC * (q + 1)],
                    NIDX, NIDX, B,
                )

                def T(j):
                    return g[:, B * j:B * (j + 1)]

                mt = gpool.tile([128, len(_COMPOSITE) * B], BF16,
                                name=f"mt_{q}", tag="mt")

                def M(mask):
                    if bin(mask).count("1") == 1:
                        return T(mask.bit_length() - 1)
                    sl = _SLOT[mask]
                    return mt[:, B * sl:B * (sl + 1)]

                for mask in _COMPOSITE:
                    low = mask & (-mask)
                    nc.vector.tensor_tensor(
                        M(mask), M(mask ^ low), T(low.bit_length() - 1),
                        mybir.AluOpType.min)

                acc = ps.tile([2, B], F32, name=f"acc_{q}", tag="acc")
                for k in range(16):
                    rhs = T(NNB) if k == 15 else M(k + 1)
                    nc.tensor.matmul(acc[:], ws_sb[:, 32 * q + 2 * k:32 * q + 2 * k + 2],
                                     rhs, start=(k == 0), stop=(k == 15))
                nc.vector.tensor_copy(stage[:, B * q:B * (q + 1)], acc[:])

            nc.sync.dma_start(out.ap().rearrange("a q b -> a (q b)"), stage[:])

    nc.compile()
    return nc


_PROG = None


def _get_program():
    global _PROG
    if _PROG is None:
        _PROG = _build_program()
    return _PROG


def _swz(a, kb):
    """[NL, kb*128, cols] -> [NL, 128, kb*cols] partition-contiguous bf16."""
    nl, rows, cols = a.shape
    assert rows == kb * 128
    return np.ascontiguousarray(
        a.reshape(nl, kb, 128, cols).transpose(0, 2, 1, 3).reshape(nl, 128, kb * cols)
    ).astype(NPBF)


def _shard_inputs(observation, action, V_W1, V_b1, V_W2, V_b2,
                  A_W1, A_b1, A_W2, A_b2, chi_m, local_edges):
    observation = np.asarray(observation, np.float32)
    action = np.asarray(action, np.float32)
    V_W1 = np.asarray(V_W1, np.float32)
    V_b1 = np.asarray(V_b1, np.float32)
    V_W2 = np.asarray(V_W2, np.float32)
    V_b2 = np.asarray(V_b2, np.float32)
    A_W1 = np.asarray(A_W1, np.float32)
    A_b1 = np.asarray(A_b1, np.float32)
    A_W2 = np.asarray(A_W2, np.float32)
    A_b2 = np.asarray(A_b2, np.float32)
    chi_m = np.asarray(chi_m, np.float32)
    local_edges = np.asarray(local_edges)

    centers = local_edges[:, 0, 0].astype(np.int64)
    neigh = local_edges[:, 0, 1:].astype(np.int64)
    csum = chi_m.sum(axis=1)  # [N, S]

    def rowbase(n):
        return int(n) * F

    in_maps = []
    for r in range(NCORES):
        nodes = np.arange(r * NL, (r + 1) * NL)
        obsT = observation[:, nodes, :].transpose(1, 2, 0)   # [NL, H, B]
        actT = action[:, nodes, :].transpose(1, 2, 0)

        # PE weight columns for the Choquet weighted sum (+ mean over F)
        wsh = np.zeros((128, NPAIR * 32), np.float32)
        # gather indices, idx i of pair q at [16g + i%16, 40q + i//16]
        gidxh = np.zeros((128, NPAIR * IDXC), np.int16)
        for q in range(NPAIR):
            n0 = int(nodes[2 * q])
            n1 = int(nodes[2 * q + 1])
            for k in range(S):
                wsh[0:64, 32 * q + 2 * k] = csum[n0, k] / (HEADS * F)
                wsh[64:128, 32 * q + 2 * k + 1] = csum[n1, k] / (HEADS * F)
            wsh[0:64, 32 * q + 30] = 1.0 / F
            wsh[64:128, 32 * q + 31] = 1.0 / F

            rows = np.empty(NIDX, np.int16)
            p = np.arange(64)
            for j in range(NNB):
                rows[128 * j:128 * j + 64] = rowbase(neigh[n0, j]) + p
                rows[128 * j + 64:128 * (j + 1)] = rowbase(neigh[n1, j]) + p
            rows[128 * NNB:128 * NNB + 64] = rowbase(centers[n0]) + p
            rows[128 * NNB + 64:128 * NG] = rowbase(centers[n1]) + p
            i = np.arange(NIDX)
            for grp in range(8):
                gidxh[16 * grp + (i % 16), IDXC * q + i // 16] = rows

        in_maps.append({
            "pk": np.ascontiguousarray(np.concatenate([
                _swz(obsT, 2), _swz(actT, 2),
                _swz(V_W1[nodes], 2), _swz(A_W1[nodes], 4),
                _swz(V_W2[nodes], 2), _swz(A_W2[nodes], 2)], axis=2)),
            "vb1T": np.ascontiguousarray(V_b1[nodes].T),
            "ab1T": np.ascontiguousarray(A_b1[nodes].T),
            "vb2T": np.ascontiguousarray(V_b2[nodes].T),
            "ab2T": np.ascontiguousarray(A_b2[nodes].T),
            "ws": wsh.astype(NPBF),
            "gidx": gidxh,
        })
    return in_maps


def _unshard_output(results):
    out = np.empty((B, N), np.float32)
    for r in range(NCORES):
        o = results[r]["out"]  # [2, NPAIR, B]
        for q in range(NPAIR):
            out[:, r * NL + 2 * q] = o[0, q]
            out[:, r * NL + 2 * q + 1] = o[1, q]
    return out


def kernel(**inputs) -> np.ndarray:
    nc = _get_program()
    in_maps = _shard_inputs(**inputs)
    results = run_bass_kernel_spmd(nc, in_maps, list(range(NCORES))).results
    return _unshard_output(results)
